# revision 39
# baseline (speedup 1.0000x reference)
"""Equivariant PQ-layer conv kernel for 8x TRN2 NeuronCores.

The layer is a 3D conv (SAME, 5^3 taps) with an assembled (320, 320, 125)
kernel over a (320, 8^3) input. The host assembles the conv kernel (cheap)
and shards the 125 taps across the 8 cores (16 tap slots per core, as 8
tap-pairs whose 640 contraction rows split into 5 exact 128-chunks).

Matmul arrangement ("transposed" vs the v1 baseline): PSUM partitions carry
voxels (512 = 4x128 chunks, no partition waste) and the free dim carries all
320 output channels (N=320 <= 512). Per pair: 5 K-chunks x 4 vox-chunks =
20 matmuls of N=320 -> 51200 charged PE rows/core vs 61440 for the
out-channels-on-partitions form (PSUM M=64 waste). lhsT (stationary) is the
shifted-input chunk [128 contraction rows, 128 voxels]; rhs (moving) is the
kernel chunk [128 contraction rows, 320 out channels].

The host pre-shifts x per tap (SPMD program must be identical across cores,
so per-core tap shifts must live in data, not in AP constants).
"""
import numpy as np
import ml_dtypes

C0, C1 = 8, 4
K = 5
G = 8
EPS = 1e-6
R_MAX = 5.5
DIM = C0 + 3 * C1          # 20
Q = 16
P = 8
NCH = DIM * Q              # 320
NV = P * P * P             # 512
K3 = K ** 3                # 125
NCORES = 8
PAIRS = 8                  # tap-pair slots per core: 8*2*8 = 128 >= 125 taps
NCC = 5                    # contraction chunks per pair: 2*320/128
NM = 4                     # vox chunks (psum banks): 512/128

LAST = None                # BassKernelResults of the most recent run
_PROGRAM = None

# PE p-state management: the TRN2 tensor engine runs at 1.2GHz until it has
# been continuously busy for 3us, then at 2.4GHz; an idle gap of ~3us resets
# the ramp (sub-1.2us gaps observed not to). Warmup fills (tiny N=8 junk
# matmuls, ~5ns each) bridge from ~t=1.3us to pair 0's data so every real
# matmul runs at full clock. Data-starved inter-pair gaps stay ~0.3us and do
# not need fills.
WARMUP = 700             # fills before pair 0's data lands
PAIR_FILLS = [0] * 8     # extra fills before each pair's wait (tuning knob)


def _levi_civita():
    e = np.zeros((3, 3, 3), np.float32)
    e[0, 1, 2] = e[1, 2, 0] = e[2, 0, 1] = 1.0
    e[0, 2, 1] = e[2, 1, 0] = e[1, 0, 2] = -1.0
    return e


def _assemble_kern(q_in, q_out, w_ss, w_vs, w_sv, w_vv0, w_vv1):
    """Mirror of the reference kernel assembly, in f32 numpy. -> (320, 320, 125)."""
    offs = np.arange(K, dtype=np.float32) - (K - 1) / 2.0
    oz, oy, ox = np.meshgrid(offs, offs, offs, indexing='ij')
    p_off = np.stack([oz, oy, ox], -1).reshape(-1, 3)
    v = p_off[None, None] - (q_out[:, None, None] - q_in[None, :, None])
    r = np.linalg.norm(v, axis=-1)
    u = np.where(r[..., None] > EPS, v / np.maximum(r, EPS)[..., None], 0.0).astype(np.float32)
    centers = np.linspace(0.0, R_MAX, G).astype(np.float32)
    sigma = R_MAX / (G - 1)
    R = np.exp(-0.5 * ((r[..., None] - centers) / sigma) ** 2).astype(np.float32)
    RY = R[..., None] * u[..., None, :]
    eye3 = np.eye(3, dtype=np.float32)
    eps3 = _levi_civita()
    K_ss = np.einsum('acg,pqkg->apcqk', w_ss, R, optimize=True)
    K_vs = np.einsum('acg,pqkgm->ampcqk', w_vs, RY, optimize=True)
    K_sv = np.einsum('acg,pqkgm->apcmqk', w_sv, RY, optimize=True)
    K_vv = (np.einsum('acg,pqkg,mn->ampcnqk', w_vv0, R, eye3, optimize=True)
            + np.float32(0.7071067811865476) *
            np.einsum('acg,pqkgm,imj->aipcjqk', w_vv1, RY, eps3, optimize=True))
    Qo, Qi = q_out.shape[0], q_in.shape[0]
    top = np.concatenate([K_ss, K_sv.reshape(C0, Qo, 3 * C1, Qi, K3)], axis=2)
    bot = np.concatenate([K_vs.reshape(3 * C1, Qo, C0, Qi, K3),
                          K_vv.reshape(3 * C1, Qo, 3 * C1, Qi, K3)], axis=2)
    kern = np.concatenate([top, bot], axis=0)
    return np.ascontiguousarray(kern.reshape(DIM * Qo, DIM * Qi, K3).astype(np.float32))


def _build_program():
    """SPMD program: 8 tap-pair slots of (kp, xs) -> partial conv output.

    Per pair p and contraction chunk c (5 chunks of 128 rows from the
    640-row tap pair): for vox chunk m: psum[m][128 vox, 320 out] +=
    xs[p][:, c*512+m*128 : +128].T @ kp[p][:, c*320 : +320].
    """
    global _PROGRAM
    if _PROGRAM is not None:
        return _PROGRAM
    from contextlib import ExitStack
    from concourse import bass, mybir

    nc = bass.Bass("TRN2", target_bir_lowering=False, debug=False,
                   enable_asserts=False, num_devices=NCORES)
    kp_d = nc.dram_tensor("kp", [128, PAIRS * NCC * NCH], mybir.dt.float16,
                          kind="ExternalInput").ap()
    # shifted input entirely in fp8e3 (values x2, kernel rows pre-halved):
    # spends accuracy margin (measured ~1.4e-2 vs the 2e-2 gate) to halve
    # the dominant stream term
    x8_d = nc.dram_tensor("x8", [128, PAIRS * NCC * NV], mybir.dt.float8e3,
                          kind="ExternalInput").ap()
    out_d = nc.dram_tensor("out_part", [128, NM * NCH], mybir.dt.float16,
                           kind="ExternalOutput").ap()

    with ExitStack() as ctx:
        kpt = ctx.enter_context(nc.sbuf_tensor("kpt", [128, PAIRS * NCC * NCH], mybir.dt.float16))
        xt8 = ctx.enter_context(nc.sbuf_tensor("xt8", [128, PAIRS * NCC * NV], mybir.dt.float8e3))
        ot = ctx.enter_context(nc.sbuf_tensor("ot", [128, NM * NCH], mybir.dt.float16))
        junk = ctx.enter_context(nc.sbuf_tensor("junk", [128, 8], mybir.dt.float16))
        ps = [ctx.enter_context(nc.psum_tensor(f"ps{m}", [128, NCH], mybir.dt.float32))
              for m in range(NM)]
        psj = ctx.enter_context(nc.psum_tensor("psj", [8, 8], mybir.dt.float32))
        xsem = ctx.enter_context(nc.semaphore("xsem"))
        msem = ctx.enter_context(nc.semaphore("msem"))
        vsem = ctx.enter_context(nc.semaphore("vsem"))
        osem = ctx.enter_context(nc.semaphore("osem"))
        jsem = ctx.enter_context(nc.semaphore("jsem"))
        wsem = ctx.enter_context(nc.semaphore("wsem"))
        block = ctx.enter_context(nc.Block())

        # kp loads on the SP HWDGE queue, x8 on the ACT queue, tiled in
        # 3-chunk pieces: large enough (>=625ns transfers) to avoid
        # HWDGE/DGE pipeline bubbles, small enough that the first piece
        # gates the PE at ~4.5us; piece pace (~1.25us) stays ahead of PE
        # consumption (~1.6us per group) so the stream never stalls compute.

        GRPS = [3] * 9 + [6, 7]
        GOFF = [sum(GRPS[:i]) for i in range(len(GRPS))]

        @block.sync
        def _(sync):
            for g0, gn in zip(GOFF, GRPS):
                sync.dma_start(out=kpt[:, g0 * NCH:(g0 + gn) * NCH],
                               in_=kp_d[:, g0 * NCH:(g0 + gn) * NCH]).then_inc(xsem, 16)
            # output drains split across queues: SP bank 0, Act bank 1,
            # Pool-SWDGE bank 2 (bypasses the contended HWDGE), Act bank 3
            # via same-queue copy+DMA. Nothing waits on osem; the program
            # ends at the last DMA's sem propagation.
            sync.wait_ge(vsem, 1)
            sync.dma_start(out=out_d[:, 0:NCH],
                           in_=ot[:, 0:NCH]).then_inc(osem, 16)

        @block.scalar
        def _(scalar):
            for g0, gn in zip(GOFF, GRPS):
                scalar.dma_start(out=xt8[:, g0 * NV:(g0 + gn) * NV],
                                 in_=x8_d[:, g0 * NV:(g0 + gn) * NV]).then_inc(xsem, 16)
            scalar.wait_ge(msem, 2)
            scalar.copy(ot[:, NCH:2 * NCH], ps[1][:, :]).then_inc(wsem, 1)
            scalar.wait_ge(wsem, 1)
            scalar.dma_start(out=out_d[:, NCH:2 * NCH],
                             in_=ot[:, NCH:2 * NCH]).then_inc(osem, 16)
            # final bank: copy and DMA on the same queue -- in-order execution
            # provides the dependency, so the DMA's descriptor-gen pipeline
            # (seq+hwdge+dge ~1.9us >> copy 450ns) overlaps the copy itself
            scalar.wait_ge(msem, 4)
            scalar.copy(ot[:, 3 * NCH:4 * NCH], ps[3][:, :]).then_inc(wsem, 1)
            scalar.dma_start(out=out_d[:, 3 * NCH:4 * NCH],
                             in_=ot[:, 3 * NCH:4 * NCH]).then_inc(osem, 16)


        @block.gpsimd
        def _(gpsimd):
            gpsimd.wait_ge(vsem, 2)
            gpsimd.dma_start(out=out_d[:, 2 * NCH:3 * NCH],
                             in_=ot[:, 2 * NCH:3 * NCH]).then_inc(osem, 16)

        @block.tensor
        def _(tensor):
            def mm(g, m, start, stop):
                return tensor.matmul(
                    ps[m][:, :],
                    xt8[:, g * NV + m * 128:g * NV + (m + 1) * 128],
                    kpt[:, g * NCH:(g + 1) * NCH],
                    start=start, stop=stop)

            def fill(n):
                for _ in range(n):
                    tensor.matmul(psj[:, :], junk[:, :], junk[:, :],
                                  start=True, stop=True)

            tensor.wait_ge(jsem, 1)
            fill(WARMUP)
            for gi, (g0, gn) in enumerate(zip(GOFF, GRPS)):
                tensor.wait_ge(xsem, 32 * (gi + 1))
                if gi < len(GRPS) - 1:
                    for g in range(g0, g0 + gn):
                        for m in range(NM):
                            mm(g, m, start=(g == 0), stop=False)
                else:
                    # m outer so psum banks complete (and drain) in order
                    for m in range(NM):
                        for g in range(g0, g0 + gn):
                            i = mm(g, m, start=False, stop=(g == g0 + gn - 1))
                            if g == g0 + gn - 1:
                                i.then_inc(msem, 1)

        @block.vector
        def _(vector):
            vector.memset(junk[:, :], 0.0).then_inc(jsem, 1)
            for m in (0, 2):
                vector.wait_ge(msem, m + 1)
                vector.tensor_copy(ot[:, m * NCH:(m + 1) * NCH], ps[m][:, :]).then_inc(vsem, 1)

    _PROGRAM = nc
    return nc


def kernel(x, q_in, q_out, w_ss, w_vs, w_sv, w_vv0, w_vv1, bias):
    global LAST
    from concourse.bass_utils import run_bass_kernel_spmd

    kern = _assemble_kern(np.asarray(q_in, np.float32), np.asarray(q_out, np.float32),
                          np.asarray(w_ss, np.float32), np.asarray(w_vs, np.float32),
                          np.asarray(w_sv, np.float32), np.asarray(w_vv0, np.float32),
                          np.asarray(w_vv1, np.float32))
    xr = np.asarray(x, np.float32).reshape(NCH, P, P, P)
    x_pad = np.zeros((NCH, P + 4, P + 4, P + 4), np.float32)
    x_pad[:, 2:10, 2:10, 2:10] = xr

    # Shifted input per tap (+1 zero slab for padding slots), fp16.
    xsh = np.zeros((K3 + 1, NCH, NV), np.float16)
    t = 0
    for dz in range(K):
        for dy in range(K):
            for dx in range(K):
                xsh[t] = x_pad[:, dz:dz + 8, dy:dy + 8, dx:dx + 8].reshape(NCH, NV)
                t += 1
    kerT = np.zeros((K3 + 1, NCH, NCH), np.float16)          # (tap, in, out)
    kerT[:K3] = kern.transpose(2, 1, 0)

    in_maps = []
    for c in range(NCORES):
        taps = list(range(c, K3, NCORES)) + [K3] * (2 * PAIRS)  # pad w/ zero slab
        taps = taps[:2 * PAIRS]
        kp_c = np.empty((PAIRS, 128, NCC * NCH), np.float16)
        x8_c = np.empty((PAIRS, 128, NCC * NV), ml_dtypes.float8_e3m4)
        for p in range(PAIRS):
            tA, tB = taps[2 * p], taps[2 * p + 1]
            kb = np.concatenate([kerT[tA], kerT[tB]], axis=0).astype(np.float32)
            kb *= 0.5                                            # fp8 x2-scale descale
            xb = np.concatenate([xsh[tA], xsh[tB]], axis=0)      # (640, 512)
            kp_c[p] = (kb.astype(np.float16)
                       .reshape(NCC, 128, NCH).transpose(1, 0, 2).reshape(128, NCC * NCH))
            x8_c[p] = ((xb.astype(np.float32) * 2.0).astype(ml_dtypes.float8_e3m4)
                       .reshape(NCC, 128, NV).transpose(1, 0, 2).reshape(128, NCC * NV))
        in_maps.append({"kp": np.ascontiguousarray(kp_c.transpose(1, 0, 2).reshape(128, -1)),
                        "x8": np.ascontiguousarray(x8_c.transpose(1, 0, 2).reshape(128, -1))})

    nc = _build_program()
    res = run_bass_kernel_spmd(nc, in_maps, list(range(NCORES)))
    LAST = res

    acc = np.zeros((128, NM * NCH), np.float64)
    for c in range(NCORES):
        acc += res.results[c]["out_part"]                    # (128, 4*320)
    out = acc.astype(np.float32).reshape(128, NM, NCH).transpose(1, 0, 2).reshape(NV, NCH)
    out = np.ascontiguousarray(out.T).reshape(1, DIM, Q, P, P, P)
    out[:, :C0] += np.asarray(bias, np.float32).reshape(1, C0, 1, 1, 1, 1)
    return out
